# revision 43
# baseline (speedup 1.0000x reference)
"""Trainium2 Bass kernel for DeiT self-attention with channel-pruning masks.

Reference computation (B=16, S=577, HID=768, H=12, D=64, N_KEEP=576):
    q/k/v = hs @ W + b            [B,S,576]
    scatter channels to [B,S,768] at {q,k,v}_idx, split into 12 heads of 64
    softmax attention per (b, h), concat heads, gather v_idx channels.

Strategy:
  - Data-parallel over batch: 8 cores x 2 images each (TOK = 1154 tokens).
  - Q/K channels packed to the per-head intersection kept_q & kept_k (only
    those columns contribute to scores), heads packed tightly into ~4
    128-row chunks -> Q/K projections cost ~4/6 of the full-width version.
    K is stored per head zero-padded over its chunk's 128 rows so score
    matmuls contract over the full chunk (zero rows null other heads).
  - Every matmul in the kernel runs in the PE's (128, 64) tile mode
    (T0 -> PSUM partitions 0-63, T1 -> 64-127).  The two tiles execute
    concurrently (HW-measured 2x), so:
      * projections/scores split output partitions in half at zero cost
        (shared rhs stream),
      * ctx matmuls pair the TWO IMAGES of a head (different rhs streams)
        into one PSUM bank -> 2x ctx throughput,
      * a single tile mode end-to-end means zero PE drain/reconfigs
        (mode switches measure ~110ns each).
  - V is packed per head (kept cols + ones column for the softmax denom,
    msizes[h] <= 64 incl. ones), so an image-pair's ctx occupies rows
    0-63 / 64-127 of one bank; one DVE cast moves both to SBUF.
  - PE and ACT (exp, ~79us floor) finish neck-and-neck; filler emitters
    are generators yielding ~<=700ns pieces so score tiles (the ACT feed)
    interleave finely with proj/V/ctx chains on the in-order PE queue.
    PSUM: 2x score tiles (4 banks) + 2 proj accums + 2 ctx banks.
  - Input DMAs are cut into small slices spread over the 3 dispatch
    queues in consumption order (a descriptor runs on a single DGE engine
    at ~22GB/s, and each dispatch serializes ~0.65us on its queue);
    a few junk matmuls warm the PE's HAM clock gate during the DMA ramp.
"""

import numpy as np

B, S, HID = 16, 577, 768
H, D = 12, 64
N_KEEP = 576
NCORES = 8
BPC = B // NCORES          # images per core
TOK = BPC * S              # tokens per core
TOKP = 1280                # token dim padded for junk-column tail reads
VW = N_KEEP + H            # 588: kept V columns + one ones column per head
VW_PAD = 624               # v_sb column pad (64-wide per-head ctx lhsT slices)
P = 128
ICH = HID // P             # 6 input-channel chunks
NK = 5
KCHUNKS = [(0, 128), (128, 128), (256, 128), (384, 128), (512, 65)]
# (q_offset, scores width, ctx width) per image; qt1 is shifted +289
QTILES = [(0, 290, 290), (289, 290, 288)]
TOK_TILES = [(0, 386), (386, 386), (772, 382)]

_NC_CACHE = {}


def _build_nc(use_f32r=True, msizes=None, qkplan=None):
    import concourse.bacc as bacc
    import concourse.mybir as mybir
    import concourse.tile as tile

    f32 = mybir.dt.float32
    mm_dt = mybir.dt.bfloat16 if use_f32r else mybir.dt.float32

    assert msizes is not None and sum(msizes) == VW
    assert all(0 < m <= 64 for m in msizes), msizes
    moffs = [sum(msizes[:h]) for h in range(H)]
    vwp = max(VW_PAD, moffs[-1] + 64)
    # qkplan: per head (chunk, row_offset, nrows); NCH packed q/k chunks
    assert qkplan is not None
    NCH = 1 + max(c for c, _, _ in qkplan)
    chunk_heads = [
        [(h, ho, mh) for h, (c, ho, mh) in enumerate(qkplan) if c == i]
        for i in range(NCH)
    ]

    nc = bacc.Bacc("TRN2", target_bir_lowering=False)

    hsT = nc.dram_tensor("hsT", [HID, TOK], mm_dt, kind="ExternalInput")
    # host-swizzled: wq[p, i, c, n] = Wq_packed[c*128+p, i*128+n]
    wq = nc.dram_tensor("wq", [P, NCH, ICH, P], mm_dt, kind="ExternalInput")
    wk = nc.dram_tensor("wk", [P, NCH, ICH, P], mm_dt, kind="ExternalInput")
    wv = nc.dram_tensor("wv", [HID, VW], mm_dt, kind="ExternalInput")
    # all f32 per-partition constants in one DMA: [bq | kmask | bkz | bvb]
    # kmask/bkz: per-head k mask (1 on the head's chunk rows) + masked k
    # bias so kz[:, h] = qp * kmask + bkz is one DVE op with an aligned
    # full-128-partition PSUM read (unaligned PSUM reads are illegal)
    NBC = NCH + 2 * H + VW
    bias = nc.dram_tensor("bias", [P, NBC], f32, kind="ExternalInput")
    outA = nc.dram_tensor("outA", [VW, TOK], mm_dt, kind="ExternalOutput")

    mm = nc.tensor.matmul
    maxh2 = max(
        len(chunk_heads[i]) + len(chunk_heads[i - 1]) for i in range(1, NCH)
    )
    EBUFS = 2 * (max(maxh2, 2 * len(chunk_heads[0]))) + 3

    with tile.TileContext(nc) as tc:
        Exp = mybir.ActivationFunctionType.Exp
        with (
            tc.tile_pool(name="big", bufs=1) as big,
            tc.tile_pool(name="psa", bufs=2, space="PSUM") as psa,   # proj accums
            tc.tile_pool(name="psb", bufs=2, space="PSUM") as psb,   # score tiles
            tc.tile_pool(name="psc", bufs=2, space="PSUM") as psc,   # ctx chains
            tc.tile_pool(name="epool", bufs=EBUFS) as epool,
            tc.tile_pool(name="opool", bufs=6) as opool,
        ):
            # ---- persistent SBUF tensors ----
            hsT_sb = big.tile([P, ICH, TOKP], mm_dt)
            hsT_r = hsT.rearrange("(c p) t -> p c t", p=P)
            wq_sb = big.tile([P, NCH, ICH, P], mm_dt)
            wk_sb = big.tile([P, NCH, ICH, P], mm_dt)
            wv_sb = big.tile([P, ICH, VW], mm_dt)
            bias_sb = big.tile([P, NBC], f32)
            bq_sb = bias_sb[:, 0:NCH]
            kmask_sb = bias_sb[:, NCH : NCH + H]
            bkz_sb = bias_sb[:, NCH + H : NCH + 2 * H]
            bvb_sb = bias_sb[:, NCH + 2 * H :]
            q_sb = big.tile([P, NCH, TOKP], mm_dt)
            # k per head, zero-padded over its chunk's 128 rows
            kz_sb = big.tile([P, H, TOKP], mm_dt)
            v_sb = big.tile([P, BPC * NK, vwp], mm_dt)

            # zero-fills on DVE (idle until the first projection bias-add):
            # kz fully (off-head rows + token tail), token tails of q/hsT
            # (read by qt1 rhs and tail-chunk junk lhsT columns), v col pad.
            nc.vector.memset(kz_sb[:, :, TOK:].bitcast(f32), 0.0)
            nc.vector.memset(q_sb[:, :, TOK:].bitcast(f32), 0.0)
            nc.vector.memset(hsT_sb[:, :, TOK:].bitcast(f32), 0.0)
            nc.vector.memset(v_sb[:, :, VW:].bitcast(f32), 0.0)

            # HAM warm-up: ~4.5us of junk matmuls on a never-written scratch
            # tile while the input DMAs land, so the PE's clock gate opens
            # (1.2 -> 2.4 GHz) before the first real chain instead of ~3.4us
            # into it. Results land in a psb slot and are never read.
            jnk = big.tile([P, 1024], mm_dt)
            nc.vector.memset(jnk[:, :].bitcast(f32), 0.0)
            jp = psb.tile([P, 1024], f32, tag="sp", name="jp")
            for w in range(6):
                half = 64 * (w % 2)
                mm(jp[half : half + 64, 0:512], jnk[:, 0:64],
                   jnk[:, 0:512], start=True, stop=True)

            # ---- input DMAs ----
            # One descriptor runs on one DGE engine at ~22GB/s, so fat loads
            # serialize; each dispatch costs ~0.65us on its queue. First-need
            # data (wq0/wk0/hsT[0:386]) is cut into small slices and spread
            # over FIVE dispatch queues (tensor and vector idle early).
            def hs_dma(q, c, a, b):
                q.dma_start(hsT_sb[:, c, a:b], hsT_r[:, c, a:b])

            hq = (nc.sync, nc.scalar, nc.gpsimd)
            for c in range(3):
                hq[c].dma_start(wq_sb[:, 0, c, :], wq[:, 0, c, :])
                hs_dma(hq[c], c, 0, 193)
                hq[c].dma_start(wq_sb[:, 0, c + 3, :], wq[:, 0, c + 3, :])
                hs_dma(hq[c], c, 193, 386)
                hs_dma(hq[c], c + 3, 0, 193)
                hs_dma(hq[c], c + 3, 193, 386)
                hq[c].dma_start(wk_sb[:, 0, 2 * c : 2 * c + 2, :],
                                wk[:, 0, 2 * c : 2 * c + 2, :])
            nc.sync.dma_start(bias_sb[:], bias[:])
            hs_dma(nc.sync, 0, 386, 772)
            hs_dma(nc.scalar, 1, 386, 772)
            hs_dma(nc.gpsimd, 2, 386, 772)
            hs_dma(nc.sync, 3, 386, 772)
            hs_dma(nc.scalar, 4, 386, 772)
            hs_dma(nc.gpsimd, 5, 386, 772)
            wv_r = wv.rearrange("(c p) n -> p c n", p=P)
            for k in range(4):
                nc.gpsimd.dma_start(wv_sb[:, k, :], wv_r[:, k, :])
            nc.sync.dma_start(wv_sb[:, 4, :], wv_r[:, 4, :])
            nc.sync.dma_start(wv_sb[:, 5, :], wv_r[:, 5, :])
            hs_dma(nc.sync, 0, 772, TOK)
            hs_dma(nc.sync, 3, 772, TOK)
            hs_dma(nc.gpsimd, 1, 772, TOK)
            hs_dma(nc.gpsimd, 4, 772, TOK)
            hs_dma(nc.gpsimd, 2, 772, TOK)
            hs_dma(nc.gpsimd, 5, 772, TOK)
            for i in range(1, NCH):
                nc.sync.dma_start(wq_sb[:, i, :, :], wq[:, i, :, :])
                nc.gpsimd.dma_start(wk_sb[:, i, :, :], wk[:, i, :, :])

            # ---- emitters ----
            # filler emitters are GENERATORS that yield after each ~<=700ns
            # piece of PE work, so the scheduler can keep score tiles (the
            # ACT feed) flowing between filler pieces on the in-order PE
            # queue. All matmuls are (128, 64) tile mode: zero PE
            # drain/reconfigs end to end.
            def gen_proj(i, w_sb, b_sb, is_k, t):
                toff, tcs = TOK_TILES[t]
                qp = psa.tile([P, 512], f32, tag="ps", name="qp")[:, :tcs]
                for k0 in range(0, ICH, 2):
                    for k in (k0, k0 + 1):
                        st, sp = (k == 0), (k == ICH - 1)
                        mm(qp[0:64, :], w_sb[:, i, k, 0:64],
                           hsT_sb[:, k, toff : toff + tcs], start=st, stop=sp)
                        mm(qp[64:128, :], w_sb[:, i, k, 64:128],
                           hsT_sb[:, k, toff : toff + tcs], start=st, stop=sp)
                    if k0 + 2 < ICH:
                        yield
                if is_k:
                    for h, ho, mh in chunk_heads[i]:
                        nc.vector.tensor_scalar(
                            out=kz_sb[:, h, toff : toff + tcs],
                            in0=qp,
                            scalar1=kmask_sb[:, h : h + 1],
                            scalar2=bkz_sb[:, h : h + 1],
                            op0=mybir.AluOpType.mult,
                            op1=mybir.AluOpType.add,
                        )
                else:
                    nc.vector.tensor_add(
                        out=q_sb[:, i, toff : toff + tcs],
                        in0=qp,
                        in1=b_sb[:, i : i + 1].to_broadcast((P, tcs)),
                    )

            VT = VW // 2  # 294

            def gen_vunit(b, j):
                koff, kcs = KCHUNKS[j]
                toff = b * S + koff
                for n in range(2):
                    vp = psa.tile([P, 512], f32, tag="ps", name="vp")[:, :VT]
                    for k0 in range(0, ICH, 2):
                        for k in (k0, k0 + 1):
                            st, sp = (k == 0), (k == ICH - 1)
                            rhs = wv_sb[:, k, n * VT : (n + 1) * VT]
                            mm(vp[0:64, :], hsT_sb[:, k, toff : toff + 64],
                               rhs, start=st, stop=sp)
                            mm(vp[64:128, :], hsT_sb[:, k, toff + 64 : toff + 128],
                               rhs, start=st, stop=sp)
                        if k0 + 2 < ICH:
                            yield
                    nc.vector.tensor_add(
                        out=v_sb[:kcs, b * NK + j, n * VT : (n + 1) * VT],
                        in0=vp[:kcs, :],
                        in1=bvb_sb[:kcs, n * VT : (n + 1) * VT],
                    )
                    if n == 0:
                        yield

            def emit_sp(i, h, b, e_sb, c):
                ko, kcs = KCHUNKS[c]
                sa = psb.tile([P, 1024], f32, tag="sp", name="sa")
                for qt, (qo, sw, cw) in enumerate(QTILES):
                    o = qt * 512
                    rhs = q_sb[:, i, b * S + qo : b * S + qo + sw]
                    mm(sa[0:64, o : o + sw],
                       kz_sb[:, h, b * S + ko : b * S + ko + 64],
                       rhs, start=True, stop=True)
                    mm(sa[64:128, o : o + sw],
                       kz_sb[:, h, b * S + ko + 64 : b * S + ko + 128],
                       rhs, start=True, stop=True)
                nc.scalar.activation(
                    e_sb[:kcs, c, :, :],
                    sa.rearrange("p (two q) -> p two q", two=2)[:kcs, :, :290],
                    Exp,
                    scale=0.125,
                )

            def gen_ctx(h, qt, e0, e1, outq=None):
                m = msizes[h]
                off = moffs[h]
                qo, sw, cw = QTILES[qt]
                cp = psc.tile([P, 512], f32, tag="cp", name="cp")[:, :cw]
                for c, (ko, kcs) in enumerate(KCHUNKS):
                    st, sp = (c == 0), (c == NK - 1)
                    mm(cp[0:64, :], v_sb[:kcs, c, off : off + 64],
                       e0[:kcs, c, qt, :cw], start=st, stop=sp)
                    mm(cp[64:128, :], v_sb[:kcs, NK + c, off : off + 64],
                       e1[:kcs, c, qt, :cw], start=st, stop=sp)
                    if c == 2:
                        yield
                o_sb = opool.tile([P, 512], mm_dt, tag="o", name="o_sb")
                nc.vector.tensor_copy(o_sb[:, :cw], cp)
                ow = cw if qt == 1 else 289
                qs = outq or (nc.gpsimd, nc.sync)
                for b in range(BPC):
                    qs[(qt + b) % 2].dma_start(
                        outA[off : off + m, b * S + qo : b * S + qo + ow],
                        o_sb[64 * b : 64 * b + m, :ow],
                    )

            # e tiles per (head, img) unit of the current + previous chunk
            es = {}

            def alloc_e(h, b):
                es[(h, b)] = epool.tile([P, NK, 2, 290], mm_dt, tag="e", name="e_sb")
                return es[(h, b)]

            # ---- piece scheduler ----
            fq = []

            def pull(n):
                k = 0
                while fq and k < n:
                    try:
                        next(fq[0])
                    except StopIteration:
                        fq.pop(0)
                    k += 1

            def drain():
                while fq:
                    try:
                        next(fq[0])
                    except StopIteration:
                        fq.pop(0)

            # ---- ramp ----
            # Q0/K0 token tiles 0,1 run first (image-0 score tiles need only
            # tokens < 772); image-0 tiles then stream while token tile 2,
            # V-projection units and Q1/K1 fill the PE.
            for g in (
                gen_proj(0, wq_sb, bq_sb, False, 0),
                gen_proj(0, wk_sb, None, True, 0),
                gen_proj(0, wq_sb, bq_sb, False, 1),
                gen_proj(0, wk_sb, None, True, 1),
            ):
                for _ in g:
                    pass

            for h, ho, mh in chunk_heads[0]:
                alloc_e(h, 0)
                alloc_e(h, 1)
            fq.append(gen_proj(0, wq_sb, bq_sb, False, 2))
            fq.append(gen_proj(0, wk_sb, None, True, 2))
            for b in range(BPC):
                for j in range(NK):
                    fq.append(gen_vunit(b, j))
            if NCH > 1:
                for t in range(3):
                    fq.append(gen_proj(1, wq_sb, bq_sb, False, t))
                    fq.append(gen_proj(1, wk_sb, None, True, t))
            s0 = [
                (h, b, c)
                for b in range(BPC)
                for c in range(NK)
                for (h, ho, mh) in chunk_heads[0]
            ]
            npieces = 6 + 6 * BPC * NK + (18 if NCH > 1 else 0)
            done = 0
            for si, (h, b, c) in enumerate(s0):
                emit_sp(0, h, b, es[(h, b)], c)
                want = (si + 1) * npieces // len(s0)
                pull(want - done)
                done = want
            drain()

            # ---- steady-state bodies: chunk i scores + chunk i-1 ctx +
            # chunk i+1 projections; last body interleaves its own ctx ----
            for i in range(1, NCH):
                last = i == NCH - 1
                for h, ho, mh in chunk_heads[i]:
                    alloc_e(h, 0)
                    alloc_e(h, 1)
                for h, ho, mh in chunk_heads[i - 1]:
                    for qt in range(2):
                        fq.append(gen_ctx(h, qt, es[(h, 0)], es[(h, 1)]))
                if not last:
                    for t in range(3):
                        fq.append(gen_proj(i + 1, wq_sb, bq_sb, False, t))
                        fq.append(gen_proj(i + 1, wk_sb, None, True, t))
                    stiles = [
                        (h, b, c)
                        for b in range(BPC)
                        for c in range(NK)
                        for (h, ho, mh) in chunk_heads[i]
                    ]
                    for si, (h, b, c) in enumerate(stiles):
                        emit_sp(i, h, b, es[(h, b)], c)
                        pull(1)
                    drain()
                else:
                    # head-major so each head's own ctx can chase its exps;
                    # final head's output DMAs avoid gpsimd (its end-of-
                    # kernel dge drain would wait on them)
                    nh = len(chunk_heads[i])
                    for hi, (h, ho, mh) in enumerate(chunk_heads[i]):
                        outq = (nc.scalar, nc.sync) if hi >= nh - 2 else None
                        for b in range(BPC):
                            for c in range(NK):
                                emit_sp(i, h, b, es[(h, b)], c)
                                pull(1)
                        for qt in range(2):
                            fq.append(
                                gen_ctx(h, qt, es[(h, 0)], es[(h, 1)], outq)
                            )
                    drain()

    nc.compile()
    return nc


def _get_nc(use_f32r=True, msizes=None, qkplan=None):
    key = ("nc", use_f32r, msizes, qkplan)
    if key not in _NC_CACHE:
        _NC_CACHE[key] = _build_nc(use_f32r, msizes, qkplan)
    return _NC_CACHE[key]


def _make_in_maps(hidden_states, Wq, bq, Wk, bk, Wv, bv, q_idx, k_idx, v_idx,
                  use_f32r=True):
    f32 = np.float32
    hs = np.asarray(hidden_states, f32)
    q_idx = np.asarray(q_idx).astype(np.int64)
    k_idx = np.asarray(k_idx).astype(np.int64)
    v_idx = np.asarray(v_idx).astype(np.int64)
    Wq = np.asarray(Wq, f32)
    Wk = np.asarray(Wk, f32)
    bqv = np.asarray(bq, f32)
    bkv = np.asarray(bk, f32)

    # per-head q&k channel intersection -> packed chunks
    qpos = {int(d): j for j, d in enumerate(q_idx)}
    kpos = {int(d): j for j, d in enumerate(k_idx)}
    qkplan = []
    cols = []            # (head, global_channel) in packed order
    cur_chunk, cur_off = 0, 0
    for h in range(H):
        ch = [d for d in range(64 * h, 64 * h + 64) if d in qpos and d in kpos]
        mh = len(ch)
        if cur_off + mh > P:
            cur_chunk += 1
            cur_off = 0
        qkplan.append((cur_chunk, cur_off, mh))
        cols.extend((h, d) for d in ch)
        cur_off += mh
    NCH = cur_chunk + 1
    wq_p = np.zeros((HID, NCH * P), f32)
    wk_p = np.zeros((HID, NCH * P), f32)
    bq_p = np.zeros(NCH * P, f32)
    kmask = np.zeros((P, H), f32)
    bkz = np.zeros((P, H), f32)
    ci = 0
    for h in range(H):
        c, ho, mh = qkplan[h]
        kmask[ho : ho + mh, h] = 1.0
        for j in range(mh):
            _, d = cols[ci]
            ci += 1
            col = c * P + ho + j
            wq_p[:, col] = Wq[:, qpos[d]]
            bq_p[col] = bqv[qpos[d]]
            wk_p[:, col] = Wk[:, kpos[d]]
            bkz[ho + j, h] = bkv[kpos[d]]

    # packed augmented V layout: per head the kept value columns (Wv columns
    # are already in sorted-v_idx order) + one ones column (softmax denom)
    Wv = np.asarray(Wv, f32)
    bv = np.asarray(bv, f32)
    kept = np.bincount(v_idx // D, minlength=H)
    msizes = tuple(int(k) + 1 for k in kept)
    wv_aug = np.zeros((HID, VW), f32)
    bv_aug = np.zeros(VW, f32)
    cum = 0
    moff = 0
    for h in range(H):
        kh = int(kept[h])
        wv_aug[:, moff : moff + kh] = Wv[:, cum : cum + kh]
        bv_aug[moff : moff + kh] = bv[cum : cum + kh]
        bv_aug[moff + kh] = 1.0
        cum += kh
        moff += kh + 1
    # combined per-partition f32 constants: [bq | kmask | bkz | bvb]
    bias_all = np.zeros((P, NCH + 2 * H + VW), f32)
    bias_all[:, 0:NCH] = bq_p.reshape(NCH, P).T
    bias_all[:, NCH : NCH + H] = kmask
    bias_all[:, NCH + H : NCH + 2 * H] = bkz
    bias_all[:, NCH + 2 * H :] = np.broadcast_to(bv_aug, (P, VW))

    if use_f32r:
        import ml_dtypes

        bf16 = ml_dtypes.bfloat16
        wq_p = wq_p.astype(bf16)
        wk_p = wk_p.astype(bf16)
        wv_aug = wv_aug.astype(bf16)
    # swizzle projection weights to [p, i, c, n] (slice-contiguous DMA layout)
    wq_p = np.ascontiguousarray(
        wq_p.reshape(ICH, P, NCH, P).transpose(1, 2, 0, 3)
    )
    wk_p = np.ascontiguousarray(
        wk_p.reshape(ICH, P, NCH, P).transpose(1, 2, 0, 3)
    )

    in_maps = []
    for c in range(NCORES):
        hsT = np.ascontiguousarray(
            hs[c * BPC : (c + 1) * BPC].reshape(TOK, HID).T
        )
        if use_f32r:
            hsT = hsT.astype(bf16)
        in_maps.append(
            {
                "hsT": hsT,
                "wq": wq_p,
                "wk": wk_p,
                "wv": wv_aug,
                "bias": bias_all,
            }
        )
    return in_maps, msizes, tuple(qkplan)


def _assemble_output(results, msizes):
    ctx = np.empty((B, S, N_KEEP), np.float32)
    vals = np.empty((N_KEEP, TOK), np.float32)
    for c in range(NCORES):
        aug = np.asarray(results[c]["outA"], np.float32)  # [VW, TOK]
        cum = 0
        moff = 0
        for h in range(H):
            kh = msizes[h] - 1
            vals[cum : cum + kh] = aug[moff : moff + kh] / aug[moff + kh]
            cum += kh
            moff += kh + 1
        ctx[c * BPC : (c + 1) * BPC] = vals.T.reshape(BPC, S, N_KEEP)
    return np.ascontiguousarray(ctx)


def run(inputs, trace=False, use_f32r=True, **spmd_kwargs):
    """Full pipeline; returns (output, BassKernelResults)."""
    from concourse import bass_utils

    in_maps, msizes, qkplan = _make_in_maps(**inputs, use_f32r=use_f32r)
    nc = _get_nc(use_f32r, msizes, qkplan)
    res = bass_utils.run_bass_kernel_spmd(
        nc, in_maps, core_ids=list(range(NCORES)), trace=trace, **spmd_kwargs
    )
    return _assemble_output(res.results, msizes), res


def kernel(**inputs):
    out, _ = run(inputs, trace=False)
    return out


# revision 44
# speedup vs baseline: 1.0054x; 1.0054x over previous
"""Trainium2 Bass kernel for DeiT self-attention with channel-pruning masks.

Reference computation (B=16, S=577, HID=768, H=12, D=64, N_KEEP=576):
    q/k/v = hs @ W + b            [B,S,576]
    scatter channels to [B,S,768] at {q,k,v}_idx, split into 12 heads of 64
    softmax attention per (b, h), concat heads, gather v_idx channels.

Strategy:
  - Data-parallel over batch: 8 cores x 2 images each (TOK = 1154 tokens).
  - Q/K channels packed to the per-head intersection kept_q & kept_k (only
    those columns contribute to scores), heads packed tightly into ~4
    128-row chunks -> Q/K projections cost ~4/6 of the full-width version.
    K is stored per head zero-padded over its chunk's 128 rows so score
    matmuls contract over the full chunk (zero rows null other heads).
  - Every matmul in the kernel runs in the PE's (128, 64) tile mode
    (T0 -> PSUM partitions 0-63, T1 -> 64-127).  The two tiles execute
    concurrently (HW-measured 2x), so:
      * projections/scores split output partitions in half at zero cost
        (shared rhs stream),
      * ctx matmuls pair the TWO IMAGES of a head (different rhs streams)
        into one PSUM bank -> 2x ctx throughput,
      * a single tile mode end-to-end means zero PE drain/reconfigs
        (mode switches measure ~110ns each).
  - V is packed per head (kept cols + ones column for the softmax denom,
    msizes[h] <= 64 incl. ones), so an image-pair's ctx occupies rows
    0-63 / 64-127 of one bank; one DVE cast moves both to SBUF.
  - PE and ACT (exp, ~79us floor) finish neck-and-neck; filler emitters
    are generators yielding ~<=700ns pieces so score tiles (the ACT feed)
    interleave finely with proj/V/ctx chains on the in-order PE queue.
    PSUM: 2x score tiles (4 banks) + 2 proj accums + 2 ctx banks.
  - Input DMAs are cut into small slices spread over the 3 dispatch
    queues in consumption order (a descriptor runs on a single DGE engine
    at ~22GB/s, and each dispatch serializes ~0.65us on its queue);
    a few junk matmuls warm the PE's HAM clock gate during the DMA ramp.
"""

import numpy as np

B, S, HID = 16, 577, 768
H, D = 12, 64
N_KEEP = 576
NCORES = 8
BPC = B // NCORES          # images per core
TOK = BPC * S              # tokens per core
TOKP = 1280                # token dim padded for junk-column tail reads
VW = N_KEEP + H            # 588: kept V columns + one ones column per head
VW_PAD = 624               # v_sb column pad (64-wide per-head ctx lhsT slices)
P = 128
ICH = HID // P             # 6 input-channel chunks
NK = 5
KCHUNKS = [(0, 128), (128, 128), (256, 128), (384, 128), (512, 65)]
# (q_offset, scores width, ctx width) per image; qt1 is shifted +289
QTILES = [(0, 290, 290), (289, 290, 288)]
TOK_TILES = [(0, 386), (386, 386), (772, 382)]

_NC_CACHE = {}


def _build_nc(use_f32r=True, msizes=None, qkplan=None):
    import concourse.bacc as bacc
    import concourse.mybir as mybir
    import concourse.tile as tile

    f32 = mybir.dt.float32
    mm_dt = mybir.dt.bfloat16 if use_f32r else mybir.dt.float32

    assert msizes is not None and sum(msizes) == VW
    assert all(0 < m <= 64 for m in msizes), msizes
    moffs = [sum(msizes[:h]) for h in range(H)]
    vwp = max(VW_PAD, moffs[-1] + 64)
    # qkplan: per head (chunk, row_offset, nrows); NCH packed q/k chunks
    assert qkplan is not None
    NCH = 1 + max(c for c, _, _ in qkplan)
    chunk_heads = [
        [(h, ho, mh) for h, (c, ho, mh) in enumerate(qkplan) if c == i]
        for i in range(NCH)
    ]

    nc = bacc.Bacc("TRN2", target_bir_lowering=False)

    hsT = nc.dram_tensor("hsT", [HID, TOK], mm_dt, kind="ExternalInput")
    # host-swizzled: wq[p, i, c, n] = Wq_packed[c*128+p, i*128+n]
    wq = nc.dram_tensor("wq", [P, NCH, ICH, P], mm_dt, kind="ExternalInput")
    wk = nc.dram_tensor("wk", [P, NCH, ICH, P], mm_dt, kind="ExternalInput")
    wv = nc.dram_tensor("wv", [HID, VW], mm_dt, kind="ExternalInput")
    # all f32 per-partition constants in one DMA: [bq | kmask | bkz | bvb]
    # kmask/bkz: per-head k mask (1 on the head's chunk rows) + masked k
    # bias so kz[:, h] = qp * kmask + bkz is one DVE op with an aligned
    # full-128-partition PSUM read (unaligned PSUM reads are illegal)
    NBC = NCH + 2 * H + VW
    bias = nc.dram_tensor("bias", [P, NBC], f32, kind="ExternalInput")
    outA = nc.dram_tensor("outA", [VW, TOK], mm_dt, kind="ExternalOutput")

    mm = nc.tensor.matmul
    maxh2 = max(
        len(chunk_heads[i]) + len(chunk_heads[i - 1]) for i in range(1, NCH)
    )
    EBUFS = 2 * (max(maxh2, 2 * len(chunk_heads[0]))) + 3

    with tile.TileContext(nc) as tc:
        Exp = mybir.ActivationFunctionType.Exp
        with (
            tc.tile_pool(name="big", bufs=1) as big,
            tc.tile_pool(name="psa", bufs=2, space="PSUM") as psa,   # proj accums
            tc.tile_pool(name="psb", bufs=2, space="PSUM") as psb,   # score tiles
            tc.tile_pool(name="psc", bufs=2, space="PSUM") as psc,   # ctx chains
            tc.tile_pool(name="epool", bufs=EBUFS) as epool,
            tc.tile_pool(name="opool", bufs=6) as opool,
        ):
            # ---- persistent SBUF tensors ----
            hsT_sb = big.tile([P, ICH, TOKP], mm_dt)
            hsT_r = hsT.rearrange("(c p) t -> p c t", p=P)
            wq_sb = big.tile([P, NCH, ICH, P], mm_dt)
            wk_sb = big.tile([P, NCH, ICH, P], mm_dt)
            wv_sb = big.tile([P, ICH, VW], mm_dt)
            bias_sb = big.tile([P, NBC], f32)
            bq_sb = bias_sb[:, 0:NCH]
            kmask_sb = bias_sb[:, NCH : NCH + H]
            bkz_sb = bias_sb[:, NCH + H : NCH + 2 * H]
            bvb_sb = bias_sb[:, NCH + 2 * H :]
            q_sb = big.tile([P, NCH, TOKP], mm_dt)
            # k per head, zero-padded over its chunk's 128 rows
            kz_sb = big.tile([P, H, TOKP], mm_dt)
            v_sb = big.tile([P, BPC * NK, vwp], mm_dt)

            # zero-fills on DVE (idle until the first projection bias-add):
            # kz fully (off-head rows + token tail), token tails of q/hsT
            # (read by qt1 rhs and tail-chunk junk lhsT columns), v col pad.
            nc.vector.memset(kz_sb[:, :, TOK:].bitcast(f32), 0.0)
            nc.vector.memset(q_sb[:, :, TOK:].bitcast(f32), 0.0)
            nc.vector.memset(hsT_sb[:, :, TOK:].bitcast(f32), 0.0)
            nc.vector.memset(v_sb[:, :, VW:].bitcast(f32), 0.0)

            # HAM warm-up: ~4.5us of junk matmuls on a never-written scratch
            # tile while the input DMAs land, so the PE's clock gate opens
            # (1.2 -> 2.4 GHz) before the first real chain instead of ~3.4us
            # into it. Results land in a psb slot and are never read.
            jnk = big.tile([P, 1024], mm_dt)
            nc.vector.memset(jnk[:, :].bitcast(f32), 0.0)
            jp = psb.tile([P, 1024], f32, tag="sp", name="jp")
            for w in range(6):
                half = 64 * (w % 2)
                mm(jp[half : half + 64, 0:512], jnk[:, 0:64],
                   jnk[:, 0:512], start=True, stop=True)

            # ---- input DMAs ----
            # One descriptor runs on one DGE engine at ~22GB/s, so fat loads
            # serialize; each dispatch costs ~0.65us on its queue. First-need
            # data (wq0/wk0/hsT[0:386]) is cut into small slices and spread
            # over FIVE dispatch queues (tensor and vector idle early).
            def hs_dma(q, c, a, b):
                q.dma_start(hsT_sb[:, c, a:b], hsT_r[:, c, a:b])

            hq = (nc.sync, nc.scalar, nc.gpsimd)
            for c in range(3):
                hq[c].dma_start(wq_sb[:, 0, c, :], wq[:, 0, c, :])
                hs_dma(hq[c], c, 0, 193)
                hq[c].dma_start(wq_sb[:, 0, c + 3, :], wq[:, 0, c + 3, :])
                hs_dma(hq[c], c, 193, 386)
                hs_dma(hq[c], c + 3, 0, 193)
                hs_dma(hq[c], c + 3, 193, 386)
                hq[c].dma_start(wk_sb[:, 0, 2 * c : 2 * c + 2, :],
                                wk[:, 0, 2 * c : 2 * c + 2, :])
            nc.sync.dma_start(bias_sb[:], bias[:])
            hs_dma(nc.sync, 0, 386, 772)
            hs_dma(nc.scalar, 1, 386, 772)
            hs_dma(nc.gpsimd, 2, 386, 772)
            hs_dma(nc.sync, 3, 386, 772)
            hs_dma(nc.scalar, 4, 386, 772)
            hs_dma(nc.gpsimd, 5, 386, 772)
            wv_r = wv.rearrange("(c p) n -> p c n", p=P)
            for k in range(4):
                nc.gpsimd.dma_start(wv_sb[:, k, :], wv_r[:, k, :])
            nc.sync.dma_start(wv_sb[:, 4, :], wv_r[:, 4, :])
            nc.sync.dma_start(wv_sb[:, 5, :], wv_r[:, 5, :])
            hs_dma(nc.sync, 0, 772, TOK)
            hs_dma(nc.sync, 3, 772, TOK)
            hs_dma(nc.gpsimd, 1, 772, TOK)
            hs_dma(nc.gpsimd, 4, 772, TOK)
            hs_dma(nc.gpsimd, 2, 772, TOK)
            hs_dma(nc.gpsimd, 5, 772, TOK)
            for i in range(1, NCH):
                nc.sync.dma_start(wq_sb[:, i, :, :], wq[:, i, :, :])
                nc.gpsimd.dma_start(wk_sb[:, i, :, :], wk[:, i, :, :])

            # ---- emitters ----
            # filler emitters are GENERATORS that yield after each ~<=700ns
            # piece of PE work, so the scheduler can keep score tiles (the
            # ACT feed) flowing between filler pieces on the in-order PE
            # queue. All matmuls are (128, 64) tile mode: zero PE
            # drain/reconfigs end to end.
            def gen_proj(i, w_sb, b_sb, is_k, t):
                toff, tcs = TOK_TILES[t]
                qp = psa.tile([P, 512], f32, tag="ps", name="qp")[:, :tcs]
                for k0 in range(0, ICH, 2):
                    for k in (k0, k0 + 1):
                        st, sp = (k == 0), (k == ICH - 1)
                        mm(qp[0:64, :], w_sb[:, i, k, 0:64],
                           hsT_sb[:, k, toff : toff + tcs], start=st, stop=sp)
                        mm(qp[64:128, :], w_sb[:, i, k, 64:128],
                           hsT_sb[:, k, toff : toff + tcs], start=st, stop=sp)
                    if k0 + 2 < ICH:
                        yield
                if is_k:
                    for h, ho, mh in chunk_heads[i]:
                        nc.vector.tensor_scalar(
                            out=kz_sb[:, h, toff : toff + tcs],
                            in0=qp,
                            scalar1=kmask_sb[:, h : h + 1],
                            scalar2=bkz_sb[:, h : h + 1],
                            op0=mybir.AluOpType.mult,
                            op1=mybir.AluOpType.add,
                        )
                else:
                    nc.vector.tensor_add(
                        out=q_sb[:, i, toff : toff + tcs],
                        in0=qp,
                        in1=b_sb[:, i : i + 1].to_broadcast((P, tcs)),
                    )

            VT = VW // 2  # 294

            def gen_vunit(b, j):
                koff, kcs = KCHUNKS[j]
                toff = b * S + koff
                for n in range(2):
                    vp = psa.tile([P, 512], f32, tag="ps", name="vp")[:, :VT]
                    for k0 in range(0, ICH, 2):
                        for k in (k0, k0 + 1):
                            st, sp = (k == 0), (k == ICH - 1)
                            rhs = wv_sb[:, k, n * VT : (n + 1) * VT]
                            mm(vp[0:64, :], hsT_sb[:, k, toff : toff + 64],
                               rhs, start=st, stop=sp)
                            mm(vp[64:128, :], hsT_sb[:, k, toff + 64 : toff + 128],
                               rhs, start=st, stop=sp)
                        if k0 + 2 < ICH:
                            yield
                    nc.vector.tensor_add(
                        out=v_sb[:kcs, b * NK + j, n * VT : (n + 1) * VT],
                        in0=vp[:kcs, :],
                        in1=bvb_sb[:kcs, n * VT : (n + 1) * VT],
                    )
                    if n == 0:
                        yield

            def emit_sp(i, h, b, e_sb, c):
                ko, kcs = KCHUNKS[c]
                sa = psb.tile([P, 1024], f32, tag="sp", name="sa")
                for qt, (qo, sw, cw) in enumerate(QTILES):
                    o = qt * 512
                    rhs = q_sb[:, i, b * S + qo : b * S + qo + sw]
                    mm(sa[0:64, o : o + sw],
                       kz_sb[:, h, b * S + ko : b * S + ko + 64],
                       rhs, start=True, stop=True)
                    mm(sa[64:128, o : o + sw],
                       kz_sb[:, h, b * S + ko + 64 : b * S + ko + 128],
                       rhs, start=True, stop=True)
                nc.scalar.activation(
                    e_sb[:kcs, c, :, :],
                    sa.rearrange("p (two q) -> p two q", two=2)[:kcs, :, :290],
                    Exp,
                    scale=0.125,
                )

            def gen_ctx(h, qt, e0, e1, outq=None):
                m = msizes[h]
                off = moffs[h]
                qo, sw, cw = QTILES[qt]
                cp = psc.tile([P, 512], f32, tag="cp", name="cp")[:, :cw]
                for c, (ko, kcs) in enumerate(KCHUNKS):
                    st, sp = (c == 0), (c == NK - 1)
                    mm(cp[0:64, :], v_sb[:kcs, c, off : off + 64],
                       e0[:kcs, c, qt, :cw], start=st, stop=sp)
                    mm(cp[64:128, :], v_sb[:kcs, NK + c, off : off + 64],
                       e1[:kcs, c, qt, :cw], start=st, stop=sp)
                    if c == 2:
                        yield
                o_sb = opool.tile([P, 512], mm_dt, tag="o", name="o_sb")
                nc.vector.tensor_copy(o_sb[:, :cw], cp)
                ow = cw if qt == 1 else 289
                qs = outq or (nc.gpsimd, nc.sync)
                for b in range(BPC):
                    qs[(qt + b) % 2].dma_start(
                        outA[off : off + m, b * S + qo : b * S + qo + ow],
                        o_sb[64 * b : 64 * b + m, :ow],
                    )

            # e tiles per (head, img) unit of the current + previous chunk
            es = {}

            def alloc_e(h, b):
                es[(h, b)] = epool.tile([P, NK, 2, 290], mm_dt, tag="e", name="e_sb")
                return es[(h, b)]

            # ---- piece scheduler ----
            fq = []

            def pull(n):
                k = 0
                while fq and k < n:
                    try:
                        next(fq[0])
                    except StopIteration:
                        fq.pop(0)
                    k += 1

            def drain():
                while fq:
                    try:
                        next(fq[0])
                    except StopIteration:
                        fq.pop(0)

            # ---- ramp ----
            # Q0/K0 token tiles 0,1 run first (image-0 score tiles need only
            # tokens < 772); image-0 tiles then stream while token tile 2,
            # V-projection units and Q1/K1 fill the PE.
            for g in (
                gen_proj(0, wq_sb, bq_sb, False, 0),
                gen_proj(0, wk_sb, None, True, 0),
                gen_proj(0, wq_sb, bq_sb, False, 1),
                gen_proj(0, wk_sb, None, True, 1),
            ):
                for _ in g:
                    pass

            for h, ho, mh in chunk_heads[0]:
                alloc_e(h, 0)
                alloc_e(h, 1)
            fq.append(gen_proj(0, wq_sb, bq_sb, False, 2))
            fq.append(gen_proj(0, wk_sb, None, True, 2))
            for b in range(BPC):
                for j in range(NK):
                    fq.append(gen_vunit(b, j))
            if NCH > 1:
                for t in range(3):
                    fq.append(gen_proj(1, wq_sb, bq_sb, False, t))
                    fq.append(gen_proj(1, wk_sb, None, True, t))
            s0 = [
                (h, b, c)
                for b in range(BPC)
                for c in range(NK)
                for (h, ho, mh) in chunk_heads[0]
            ]
            npieces = 6 + 6 * BPC * NK + (18 if NCH > 1 else 0)
            done = 0
            for si, (h, b, c) in enumerate(s0):
                emit_sp(0, h, b, es[(h, b)], c)
                want = (si + 1) * npieces // len(s0)
                pull(want - done)
                done = want
            drain()

            # ---- steady-state bodies: chunk i scores + chunk i-1 ctx +
            # chunk i+1 projections; last body interleaves its own ctx ----
            for i in range(1, NCH):
                last = i == NCH - 1
                for h, ho, mh in chunk_heads[i]:
                    alloc_e(h, 0)
                    alloc_e(h, 1)
                for h, ho, mh in chunk_heads[i - 1]:
                    for qt in range(2):
                        fq.append(gen_ctx(h, qt, es[(h, 0)], es[(h, 1)]))
                if not last:
                    for t in range(3):
                        fq.append(gen_proj(i + 1, wq_sb, bq_sb, False, t))
                        fq.append(gen_proj(i + 1, wk_sb, None, True, t))
                    stiles = [
                        (h, b, c)
                        for b in range(BPC)
                        for c in range(NK)
                        for (h, ho, mh) in chunk_heads[i]
                    ]
                    for si, (h, b, c) in enumerate(stiles):
                        emit_sp(i, h, b, es[(h, b)], c)
                        pull(1)
                    drain()
                else:
                    # head-major so each head's own ctx can chase its exps;
                    # final head's output DMAs avoid gpsimd (its end-of-
                    # kernel dge drain would wait on them)
                    nh = len(chunk_heads[i])
                    for hi, (h, ho, mh) in enumerate(chunk_heads[i]):
                        outq = (nc.scalar, nc.sync) if hi == nh - 1 else None
                        for b in range(BPC):
                            for c in range(NK):
                                emit_sp(i, h, b, es[(h, b)], c)
                                pull(1)
                        for qt in range(2):
                            fq.append(
                                gen_ctx(h, qt, es[(h, 0)], es[(h, 1)], outq)
                            )
                    drain()

    nc.compile()
    return nc


def _get_nc(use_f32r=True, msizes=None, qkplan=None):
    key = ("nc", use_f32r, msizes, qkplan)
    if key not in _NC_CACHE:
        _NC_CACHE[key] = _build_nc(use_f32r, msizes, qkplan)
    return _NC_CACHE[key]


def _make_in_maps(hidden_states, Wq, bq, Wk, bk, Wv, bv, q_idx, k_idx, v_idx,
                  use_f32r=True):
    f32 = np.float32
    hs = np.asarray(hidden_states, f32)
    q_idx = np.asarray(q_idx).astype(np.int64)
    k_idx = np.asarray(k_idx).astype(np.int64)
    v_idx = np.asarray(v_idx).astype(np.int64)
    Wq = np.asarray(Wq, f32)
    Wk = np.asarray(Wk, f32)
    bqv = np.asarray(bq, f32)
    bkv = np.asarray(bk, f32)

    # per-head q&k channel intersection -> packed chunks
    qpos = {int(d): j for j, d in enumerate(q_idx)}
    kpos = {int(d): j for j, d in enumerate(k_idx)}
    qkplan = []
    cols = []            # (head, global_channel) in packed order
    cur_chunk, cur_off = 0, 0
    for h in range(H):
        ch = [d for d in range(64 * h, 64 * h + 64) if d in qpos and d in kpos]
        mh = len(ch)
        if cur_off + mh > P:
            cur_chunk += 1
            cur_off = 0
        qkplan.append((cur_chunk, cur_off, mh))
        cols.extend((h, d) for d in ch)
        cur_off += mh
    NCH = cur_chunk + 1
    wq_p = np.zeros((HID, NCH * P), f32)
    wk_p = np.zeros((HID, NCH * P), f32)
    bq_p = np.zeros(NCH * P, f32)
    kmask = np.zeros((P, H), f32)
    bkz = np.zeros((P, H), f32)
    ci = 0
    for h in range(H):
        c, ho, mh = qkplan[h]
        kmask[ho : ho + mh, h] = 1.0
        for j in range(mh):
            _, d = cols[ci]
            ci += 1
            col = c * P + ho + j
            wq_p[:, col] = Wq[:, qpos[d]]
            bq_p[col] = bqv[qpos[d]]
            wk_p[:, col] = Wk[:, kpos[d]]
            bkz[ho + j, h] = bkv[kpos[d]]

    # packed augmented V layout: per head the kept value columns (Wv columns
    # are already in sorted-v_idx order) + one ones column (softmax denom)
    Wv = np.asarray(Wv, f32)
    bv = np.asarray(bv, f32)
    kept = np.bincount(v_idx // D, minlength=H)
    msizes = tuple(int(k) + 1 for k in kept)
    wv_aug = np.zeros((HID, VW), f32)
    bv_aug = np.zeros(VW, f32)
    cum = 0
    moff = 0
    for h in range(H):
        kh = int(kept[h])
        wv_aug[:, moff : moff + kh] = Wv[:, cum : cum + kh]
        bv_aug[moff : moff + kh] = bv[cum : cum + kh]
        bv_aug[moff + kh] = 1.0
        cum += kh
        moff += kh + 1
    # combined per-partition f32 constants: [bq | kmask | bkz | bvb]
    bias_all = np.zeros((P, NCH + 2 * H + VW), f32)
    bias_all[:, 0:NCH] = bq_p.reshape(NCH, P).T
    bias_all[:, NCH : NCH + H] = kmask
    bias_all[:, NCH + H : NCH + 2 * H] = bkz
    bias_all[:, NCH + 2 * H :] = np.broadcast_to(bv_aug, (P, VW))

    if use_f32r:
        import ml_dtypes

        bf16 = ml_dtypes.bfloat16
        wq_p = wq_p.astype(bf16)
        wk_p = wk_p.astype(bf16)
        wv_aug = wv_aug.astype(bf16)
    # swizzle projection weights to [p, i, c, n] (slice-contiguous DMA layout)
    wq_p = np.ascontiguousarray(
        wq_p.reshape(ICH, P, NCH, P).transpose(1, 2, 0, 3)
    )
    wk_p = np.ascontiguousarray(
        wk_p.reshape(ICH, P, NCH, P).transpose(1, 2, 0, 3)
    )

    in_maps = []
    for c in range(NCORES):
        hsT = np.ascontiguousarray(
            hs[c * BPC : (c + 1) * BPC].reshape(TOK, HID).T
        )
        if use_f32r:
            hsT = hsT.astype(bf16)
        in_maps.append(
            {
                "hsT": hsT,
                "wq": wq_p,
                "wk": wk_p,
                "wv": wv_aug,
                "bias": bias_all,
            }
        )
    return in_maps, msizes, tuple(qkplan)


def _assemble_output(results, msizes):
    ctx = np.empty((B, S, N_KEEP), np.float32)
    vals = np.empty((N_KEEP, TOK), np.float32)
    for c in range(NCORES):
        aug = np.asarray(results[c]["outA"], np.float32)  # [VW, TOK]
        cum = 0
        moff = 0
        for h in range(H):
            kh = msizes[h] - 1
            vals[cum : cum + kh] = aug[moff : moff + kh] / aug[moff + kh]
            cum += kh
            moff += kh + 1
        ctx[c * BPC : (c + 1) * BPC] = vals.T.reshape(BPC, S, N_KEEP)
    return np.ascontiguousarray(ctx)


def run(inputs, trace=False, use_f32r=True, **spmd_kwargs):
    """Full pipeline; returns (output, BassKernelResults)."""
    from concourse import bass_utils

    in_maps, msizes, qkplan = _make_in_maps(**inputs, use_f32r=use_f32r)
    nc = _get_nc(use_f32r, msizes, qkplan)
    res = bass_utils.run_bass_kernel_spmd(
        nc, in_maps, core_ids=list(range(NCORES)), trace=trace, **spmd_kwargs
    )
    return _assemble_output(res.results, msizes), res


def kernel(**inputs):
    out, _ = run(inputs, trace=False)
    return out
